# revision 15
# baseline (speedup 1.0000x reference)
"""NeocortexMemory (vq_codebook) Trainium2 kernel.

Reference computation (B=65536, D=1024, P=32):
    h    = x @ Wi.T + bi
    sim  = (h/||h||) @ (proto/||proto||).T / 0.1
    attn = softmax(sim)
    out  = gelu([x, attn @ proto] @ Wo.T + bo)
    y    = LayerNorm(out + x) * gamma + beta

Device strategy (pure data-parallel over B, 8 cores x 8192 rows):
  Host folds:
    Q  = Wi.T @ proto_norm.T   [D, P]   -> s_un = x @ Q + c (c = proto_norm @ bi)
    Mp = proto @ Wo[:,D:].T + bo [P, D] -> out2 = x @ Wox.T + attn @ Mp
  so the only large matmuls are h = x@Wi.T (needed for ||h|| only) and
  x @ Wox.T; the retrieved-path contraction drops from K=1024 to K=32.

  Per 512-row supertile, feature-on-partitions orientation:
    xT chunks via PE transpose; h.T accumulated in PSUM per 128-feature tile,
    squared (+bi) on ACT, partition-summed via ones-matmul -> ||h||^2 [1,512];
    s_un.T = Q-chunks x xT [32,512]; softmax across 32 partitions using
    ones-matmul for the sum and SBUF->SBUF DMA row-broadcasts for the
    per-column scales; out2 accumulated per 128-row tile in [row, feature]
    orientation (stationary = xT/attnT columns), gelu on ACT straight out of
    PSUM, residual add + LayerNorm (bn_stats/bn_aggr) on DVE, DMA out.
"""

import numpy as np

import concourse.bass as bass
import concourse.bacc as bacc
import concourse.tile as tile
import concourse.mybir as mybir
from concourse.bass_utils import run_bass_kernel_spmd
from concourse.masks import make_identity

N_CORES = 8
B, D, P = 65536, 1024, 32
BL = B // N_CORES          # rows per core
SB = 512                   # supertile rows
N_ST = BL // SB            # supertiles per core
N_BT = SB // 128           # 128-row tiles per supertile
N_CH = D // 128            # 128-wide feature chunks
TEMP = 0.1
EPS_LN = 1e-5
F32 = mybir.dt.float32
F32R = mybir.dt.float32r
AF = mybir.ActivationFunctionType
ALU = mybir.AluOpType

# matmul input dtype: float32r streams at 1 cycle/row (N>=256) vs 4 for
# float32. Tiles feeding matmuls are declared float32r so their producers
# round into the format (BIR verifier requirement).


def build_kernel():
    nc = bacc.Bacc("TRN2", target_bir_lowering=False, debug=False)

    x_d = nc.dram_tensor("x", [BL, D], F32, kind="ExternalInput")
    wit_d = nc.dram_tensor("wit", [128, N_CH, D], F32R, kind="ExternalInput")
    woxt_d = nc.dram_tensor("woxt", [128, N_CH, D], F32R, kind="ExternalInput")
    q_d = nc.dram_tensor("q", [128, N_CH, P], F32R, kind="ExternalInput")
    mp_d = nc.dram_tensor("mp", [P, D], F32R, kind="ExternalInput")
    bi_d = nc.dram_tensor("bi", [128, N_CH], F32, kind="ExternalInput")
    c_d = nc.dram_tensor("cvec", [P, 1], F32, kind="ExternalInput")
    ones_d = nc.dram_tensor("ones", [128, P], F32R, kind="ExternalInput")
    out_d = nc.dram_tensor("out", [BL, D], F32, kind="ExternalOutput")

    with tile.TileContext(nc) as tc:
        with (
            tc.tile_pool(name="singles", bufs=1) as singles,
            tc.tile_pool(name="xp", bufs=2 * N_BT) as xp,
            tc.tile_pool(name="xtp", bufs=2 * N_CH) as xtp,
            tc.tile_pool(name="hsqp", bufs=2) as hsqp,
            tc.tile_pool(name="smallp", bufs=3) as smallp,
            tc.tile_pool(name="yp", bufs=3) as yp,
            tc.tile_pool(name="statp", bufs=4) as statp,
            tc.tile_pool(name="ps_h", bufs=2, space="PSUM") as ps_h,
            tc.tile_pool(name="ps_tr", bufs=2, space="PSUM") as ps_tr,
            tc.tile_pool(name="ps_o", bufs=2, space="PSUM") as ps_o,
            tc.tile_pool(name="ps_s", bufs=2, space="PSUM") as ps_s,
        ):
            # ---- one-time weight loads ----
            wit_sb = singles.tile([128, N_CH, D], F32R)
            nc.sync.dma_start(out=wit_sb, in_=wit_d[:])
            woxt_sb = singles.tile([128, N_CH, D], F32R)
            nc.sync.dma_start(out=woxt_sb, in_=woxt_d[:])
            q_sb = singles.tile([128, N_CH, P], F32R)
            nc.sync.dma_start(out=q_sb, in_=q_d[:])
            mp_sb = singles.tile([P, D], F32R)
            nc.sync.dma_start(out=mp_sb, in_=mp_d[:])
            bi_sb = singles.tile([128, N_CH], F32)
            nc.sync.dma_start(out=bi_sb, in_=bi_d[:])
            c_sb = singles.tile([P, 1], F32)
            nc.sync.dma_start(out=c_sb, in_=c_d[:])
            ident = singles.tile([128, 128], F32)
            make_identity(nc, ident)
            # ones with M=P columns: the partition-sum matmuls then write the
            # same sum to all P output partitions — broadcast for free.
            ones_sb = singles.tile([128, P], F32R)
            nc.sync.dma_start(out=ones_sb, in_=ones_d[:])
            ones_mP = ones_sb
            ones_z = ones_sb[0:P, :]
            eps_sb = singles.tile([128, 1], F32)
            nc.vector.memset(eps_sb, EPS_LN)

            for st in range(N_ST):
                r0 = st * SB

                # ---- load x tiles [128, D] ----
                x_t = []
                for t in range(N_BT):
                    xt = xp.tile([128, D], F32, tag="x")
                    nc.sync.dma_start(
                        out=xt, in_=x_d[r0 + t * 128 : r0 + (t + 1) * 128, :]
                    )
                    x_t.append(xt)

                # ---- transpose x -> xT chunks [128 d, SB b] ----
                xT = []
                for c in range(N_CH):
                    trp = ps_tr.tile([128, SB], F32, tag="tr")
                    for t in range(N_BT):
                        nc.tensor.transpose(
                            out=trp[:, t * 128 : (t + 1) * 128],
                            in_=x_t[t][:, c * 128 : (c + 1) * 128],
                            identity=ident,
                        )
                    xc = xtp.tile([128, SB], F32R, tag="xT")
                    nc.vector.tensor_copy(out=xc, in_=trp)
                    xT.append(xc)

                # ---- h.T per feature tile; square(+bi); accumulate ||h||^2 ----
                hsq = []
                for jt in range(N_CH):
                    hp = ps_h.tile([128, SB], F32, tag="h")
                    for c in range(N_CH):
                        nc.tensor.matmul(
                            out=hp,
                            lhsT=wit_sb[:, c, jt * 128 : (jt + 1) * 128],
                            rhs=xT[c],
                            start=(c == 0),
                            stop=(c == N_CH - 1),
                        )
                    hs = hsqp.tile([128, SB], F32R, tag="hsq")
                    nc.scalar.activation(
                        out=hs, in_=hp, func=AF.Square, bias=bi_sb[:, jt : jt + 1]
                    )
                    hsq.append(hs)

                # s_un.T [P, SB] = sum_c Q_c.T @ xT_c
                sun_ps = ps_s.tile([P, SB], F32, tag="small")
                for c in range(N_CH):
                    nc.tensor.matmul(
                        out=sun_ps,
                        lhsT=q_sb[:, c, :],
                        rhs=xT[c],
                        start=(c == 0),
                        stop=(c == N_CH - 1),
                    )

                # ||h||^2 broadcast to all P partitions via ones-matmul
                nrm_ps = ps_s.tile([P, SB], F32, tag="small")
                for jt in range(N_CH):
                    nc.tensor.matmul(
                        out=nrm_ps,
                        lhsT=ones_mP,
                        rhs=hsq[jt],
                        start=(jt == 0),
                        stop=(jt == N_CH - 1),
                    )

                # rn = 10 / ||h||  (sqrt(normsq/100) then reciprocal)
                hn = smallp.tile([P, SB], F32, tag="hn")
                nc.scalar.activation(
                    out=hn, in_=nrm_ps, func=AF.Sqrt, scale=TEMP * TEMP
                )
                rn = smallp.tile([P, SB], F32, tag="rn")
                nc.vector.reciprocal(out=rn, in_=hn)

                # sim.T = (s_un + c) * rn  -> exp
                sT = smallp.tile([P, SB], F32, tag="sT")
                nc.vector.tensor_scalar(
                    out=sT, in0=sun_ps, scalar1=c_sb, scalar2=None, op0=ALU.add
                )
                simT = smallp.tile([P, SB], F32, tag="simT")
                nc.vector.tensor_mul(out=simT, in0=sT, in1=rn)
                expT = smallp.tile([P, SB], F32R, tag="expT")
                nc.scalar.activation(out=expT, in_=simT, func=AF.Exp)

                # Z = sum_p exp (broadcast to all P partitions) -> 1/Z -> attn.T
                z_ps = ps_s.tile([P, SB], F32, tag="small")
                nc.tensor.matmul(
                    out=z_ps,
                    lhsT=ones_z,
                    rhs=expT,
                    start=True,
                    stop=True,
                )
                rz = smallp.tile([P, SB], F32, tag="rz")
                nc.vector.reciprocal(out=rz, in_=z_ps)
                attnT = smallp.tile([P, SB], F32R, tag="attnT")
                nc.vector.tensor_mul(out=attnT, in0=expT.bitcast(F32), in1=rz)

                # ---- out2 = x @ Wox.T + attn @ Mp ; gelu; +x; layernorm ----
                for t in range(N_BT):
                    yt = yp.tile([128, D], F32, tag="y")
                    for jh in range(2):
                        op = ps_o.tile([128, 512], F32, tag="o")
                        for c in range(N_CH):
                            nc.tensor.matmul(
                                out=op,
                                lhsT=xT[c][:, t * 128 : (t + 1) * 128],
                                rhs=woxt_sb[:, c, jh * 512 : (jh + 1) * 512],
                                start=(c == 0),
                                stop=False,
                            )
                        nc.tensor.matmul(
                            out=op,
                            lhsT=attnT[:, t * 128 : (t + 1) * 128],
                            rhs=mp_sb[:, jh * 512 : (jh + 1) * 512],
                            start=False,
                            stop=True,
                        )
                        nc.scalar.activation(
                            out=yt[:, jh * 512 : (jh + 1) * 512], in_=op, func=AF.Gelu
                        )
                    nc.vector.tensor_add(out=yt, in0=yt, in1=x_t[t])

                    stats = statp.tile([128, 2, 6], F32, tag="stats")
                    yr = yt.rearrange("p (s f) -> p s f", f=512)
                    for s in range(2):
                        nc.vector.bn_stats(out=stats[:, s, :], in_=yr[:, s, :])
                    mv = statp.tile([128, 2], F32, tag="mv")
                    nc.vector.bn_aggr(out=mv, in_=stats)
                    sd = statp.tile([128, 1], F32, tag="sd")
                    nc.scalar.activation(
                        out=sd, in_=mv[:, 1:2], func=AF.Sqrt, bias=eps_sb
                    )
                    rstd = statp.tile([128, 1], F32, tag="rstd")
                    nc.vector.reciprocal(out=rstd, in_=sd)
                    nc.vector.tensor_scalar(
                        out=yt,
                        in0=yt,
                        scalar1=mv[:, 0:1],
                        scalar2=rstd,
                        op0=ALU.subtract,
                        op1=ALU.mult,
                    )
                    nc.sync.dma_start(
                        out=out_d[r0 + t * 128 : r0 + (t + 1) * 128, :], in_=yt
                    )

    nc.finalize()
    return nc


_RUNNER_CACHE = None


def _get_runner():
    """Build the bass program once and wrap it in a cached jitted callable
    (same lowering as bass2jax.run_bass_via_pjrt, reusable across calls)."""
    global _RUNNER_CACHE
    if _RUNNER_CACHE is not None:
        return _RUNNER_CACHE

    import jax
    from jax.sharding import Mesh, PartitionSpec
    from jax.experimental.shard_map import shard_map
    import concourse.mybir as mb
    from concourse import bass2jax

    nc = build_kernel()
    bass2jax.install_neuronx_cc_hook()

    partition_name = (
        nc.partition_id_tensor.name if nc.partition_id_tensor else None
    )
    in_names, out_names, out_avals, zero_outs = [], [], [], []
    for alloc in nc.m.functions[0].allocations:
        if not isinstance(alloc, mb.MemoryLocationSet):
            continue
        name = alloc.memorylocations[0].name
        if alloc.kind == "ExternalInput":
            if name != partition_name:
                in_names.append(name)
        elif alloc.kind == "ExternalOutput":
            shape = tuple(alloc.tensor_shape)
            dtype = mb.dt.np(alloc.dtype)
            out_names.append(name)
            out_avals.append(jax.core.ShapedArray(shape, dtype))
            zero_outs.append(np.zeros(shape, dtype))
    n_params = len(in_names)
    all_in_names = in_names + out_names
    if partition_name is not None:
        all_in_names = all_in_names + [partition_name]

    def _body(*args):
        operands = list(args)
        if partition_name is not None:
            operands.append(bass2jax.partition_id_tensor())
        outs = bass2jax._bass_exec_p.bind(
            *operands,
            out_avals=tuple(out_avals),
            in_names=tuple(all_in_names),
            out_names=tuple(out_names),
            lowering_input_output_aliases=(),
            sim_require_finite=True,
            sim_require_nnan=True,
            nc=nc,
        )
        return tuple(outs)

    devices = jax.devices()[:N_CORES]
    mesh = Mesh(np.asarray(devices), ("core",))
    specs = (PartitionSpec("core"),) * (n_params + len(out_names))
    fn = jax.jit(
        shard_map(
            _body,
            mesh=mesh,
            in_specs=specs,
            out_specs=(PartitionSpec("core"),) * len(out_names),
            check_rep=False,
        ),
        donate_argnums=tuple(range(n_params, n_params + len(out_names))),
        keep_unused=True,
    )
    _RUNNER_CACHE = (fn, in_names, out_names, out_avals, zero_outs, mesh)
    return _RUNNER_CACHE


def _host_prep(Wi, bi, Wo, bo, gamma, beta, prototypes):
    f64 = np.float64
    pn = prototypes.astype(f64)
    pn = pn / np.maximum(np.linalg.norm(pn, axis=-1, keepdims=True), 1e-12)
    Q = (Wi.astype(f64).T @ pn.T).astype(np.float32)          # [D, P]
    cvec = (pn @ bi.astype(f64)).astype(np.float32)           # [P]
    Mp = (
        prototypes.astype(f64) @ Wo.astype(f64)[:, D:].T + bo.astype(f64)[None, :]
    ).astype(np.float32)                                       # [P, D]
    WiT = np.ascontiguousarray(Wi.T)                           # [D, D]
    WoxT = np.ascontiguousarray(Wo[:, :D].T)                   # [D, D]

    def chunked(w, width):
        # [D, width] -> [128, N_CH, width] with row d = c*128 + p at [p, c]
        return np.ascontiguousarray(
            w.reshape(N_CH, 128, width).transpose(1, 0, 2)
        )

    return {
        "wit": chunked(WiT, D),
        "woxt": chunked(WoxT, D),
        "q": chunked(Q, P),
        "mp": np.ascontiguousarray(Mp),
        "bi": np.ascontiguousarray(bi.reshape(N_CH, 128).T),
        "cvec": np.ascontiguousarray(cvec.reshape(P, 1)),
        "ones": np.ones((128, P), np.float32),
    }


def _concat_inputs(x, weights):
    """Build the concatenated (n_cores*dim0) operand list in in_names order."""
    fn, in_names, out_names, out_avals, zero_outs, mesh = _get_runner()
    per_core_x = x.reshape(N_CORES, BL, D)
    ops = []
    for name in in_names:
        if name == "x":
            ops.append(np.ascontiguousarray(per_core_x.reshape(N_CORES * BL, D)))
        else:
            w = weights[name]
            ops.append(
                np.ascontiguousarray(
                    np.broadcast_to(w[None], (N_CORES, *w.shape)).reshape(
                        N_CORES * w.shape[0], *w.shape[1:]
                    )
                )
            )
    zeros = [
        np.zeros((N_CORES * z.shape[0], *z.shape[1:]), z.dtype) for z in zero_outs
    ]
    return ops, zeros


def kernel(x, Wi, bi, Wo, bo, gamma, beta, prototypes):
    x = np.ascontiguousarray(np.asarray(x, dtype=np.float32))
    weights = _host_prep(
        np.asarray(Wi, np.float32), np.asarray(bi, np.float32),
        np.asarray(Wo, np.float32), np.asarray(bo, np.float32),
        np.asarray(gamma, np.float32), np.asarray(beta, np.float32),
        np.asarray(prototypes, np.float32),
    )
    fn, in_names, out_names, out_avals, zero_outs, mesh = _get_runner()
    ops, zeros = _concat_inputs(x, weights)
    out_arrs = fn(*ops, *zeros)
    out = np.asarray(out_arrs[out_names.index("out")])
    # gamma/beta are identity in this problem's setup; guard anyway.
    gamma = np.asarray(gamma, np.float32)
    beta = np.asarray(beta, np.float32)
    if not (np.all(gamma == 1.0) and np.all(beta == 0.0)):
        out = out * gamma[None, :] + beta[None, :]
    return out


# revision 17
# speedup vs baseline: 62.8277x; 62.8277x over previous
"""NeocortexMemory (vq_codebook) Trainium2 kernel.

Reference computation (B=65536, D=1024, P=32):
    h    = x @ Wi.T + bi
    sim  = (h/||h||) @ (proto/||proto||).T / 0.1
    attn = softmax(sim)
    out  = gelu([x, attn @ proto] @ Wo.T + bo)
    y    = LayerNorm(out + x) * gamma + beta

Device strategy (pure data-parallel over B, 8 cores x 8192 rows):
  Host folds:
    Q  = Wi.T @ proto_norm.T   [D, P]   -> s_un = x @ Q + c (c = proto_norm @ bi)
    Mp = proto @ Wo[:,D:].T + bo [P, D] -> out2 = x @ Wox.T + attn @ Mp
  so the only large matmuls are h = x@Wi.T (needed for ||h|| only) and
  x @ Wox.T; the retrieved-path contraction drops from K=1024 to K=32.

  Per 512-row supertile, feature-on-partitions orientation:
    xT chunks via PE transpose; h.T accumulated in PSUM per 128-feature tile,
    squared (+bi) on ACT, partition-summed via ones-matmul -> ||h||^2 [1,512];
    s_un.T = Q-chunks x xT [32,512]; softmax across 32 partitions using
    ones-matmul for the sum and SBUF->SBUF DMA row-broadcasts for the
    per-column scales; out2 accumulated per 128-row tile in [row, feature]
    orientation (stationary = xT/attnT columns), gelu on ACT straight out of
    PSUM, residual add + LayerNorm (bn_stats/bn_aggr) on DVE, DMA out.
"""

import numpy as np

import concourse.bass as bass
import concourse.bacc as bacc
import concourse.tile as tile
import concourse.mybir as mybir
from concourse.bass_utils import run_bass_kernel_spmd
from concourse.masks import make_identity

N_CORES = 8
B, D, P = 65536, 1024, 32
BL = B // N_CORES          # rows per core
SB = 512                   # supertile rows
N_ST = BL // SB            # supertiles per core
N_BT = SB // 128           # 128-row tiles per supertile
N_CH = D // 128            # 128-wide feature chunks
TEMP = 0.1
EPS_LN = 1e-5
F32 = mybir.dt.float32
F32R = mybir.dt.float32r
AF = mybir.ActivationFunctionType
ALU = mybir.AluOpType

# matmul input dtype: float32r streams at 1 cycle/row (N>=256) vs 4 for
# float32. Tiles feeding matmuls are declared float32r so their producers
# round into the format (BIR verifier requirement).


def build_kernel(reps=1):
    nc = bacc.Bacc("TRN2", target_bir_lowering=False, debug=False)

    x_d = nc.dram_tensor("x", [BL, D], F32, kind="ExternalInput")
    wit_d = nc.dram_tensor("wit", [128, N_CH, D], F32R, kind="ExternalInput")
    woxt_d = nc.dram_tensor("woxt", [128, N_CH, D], F32R, kind="ExternalInput")
    q_d = nc.dram_tensor("q", [128, N_CH, P], F32R, kind="ExternalInput")
    mp_d = nc.dram_tensor("mp", [P, D], F32R, kind="ExternalInput")
    bi_d = nc.dram_tensor("bi", [128, N_CH], F32, kind="ExternalInput")
    c_d = nc.dram_tensor("cvec", [P, 1], F32, kind="ExternalInput")
    ones_d = nc.dram_tensor("ones", [128, P], F32R, kind="ExternalInput")
    out_d = nc.dram_tensor("out", [BL, D], F32, kind="ExternalOutput")

    with tile.TileContext(nc) as tc:
        with (
            tc.tile_pool(name="singles", bufs=1) as singles,
            tc.tile_pool(name="xp", bufs=2 * N_BT) as xp,
            tc.tile_pool(name="xtp", bufs=2 * N_CH) as xtp,
            tc.tile_pool(name="hsqp", bufs=2) as hsqp,
            tc.tile_pool(name="smallp", bufs=3) as smallp,
            tc.tile_pool(name="yp", bufs=3) as yp,
            tc.tile_pool(name="statp", bufs=4) as statp,
            tc.tile_pool(name="ps_h", bufs=2, space="PSUM") as ps_h,
            tc.tile_pool(name="ps_tr", bufs=2, space="PSUM") as ps_tr,
            tc.tile_pool(name="ps_o", bufs=2, space="PSUM") as ps_o,
            tc.tile_pool(name="ps_s", bufs=2, space="PSUM") as ps_s,
        ):
            # ---- one-time weight loads ----
            wit_sb = singles.tile([128, N_CH, D], F32R)
            nc.sync.dma_start(out=wit_sb, in_=wit_d[:])
            woxt_sb = singles.tile([128, N_CH, D], F32R)
            nc.sync.dma_start(out=woxt_sb, in_=woxt_d[:])
            q_sb = singles.tile([128, N_CH, P], F32R)
            nc.sync.dma_start(out=q_sb, in_=q_d[:])
            mp_sb = singles.tile([P, D], F32R)
            nc.sync.dma_start(out=mp_sb, in_=mp_d[:])
            bi_sb = singles.tile([128, N_CH], F32)
            nc.sync.dma_start(out=bi_sb, in_=bi_d[:])
            c_sb = singles.tile([P, 1], F32)
            nc.sync.dma_start(out=c_sb, in_=c_d[:])
            ident = singles.tile([128, 128], F32)
            make_identity(nc, ident)
            # ones with M=P columns: the partition-sum matmuls then write the
            # same sum to all P output partitions — broadcast for free.
            ones_sb = singles.tile([128, P], F32R)
            nc.sync.dma_start(out=ones_sb, in_=ones_d[:])
            ones_mP = ones_sb
            ones_z = ones_sb[0:P, :]
            eps_sb = singles.tile([128, 1], F32)
            nc.vector.memset(eps_sb, EPS_LN)

            for rep in range(reps):
              for st in range(N_ST):
                r0 = st * SB

                # ---- load x tiles [128, D] ----
                x_t = []
                for t in range(N_BT):
                    xt = xp.tile([128, D], F32, tag="x")
                    nc.sync.dma_start(
                        out=xt, in_=x_d[r0 + t * 128 : r0 + (t + 1) * 128, :]
                    )
                    x_t.append(xt)

                # ---- transpose x -> xT chunks [128 d, SB b] ----
                xT = []
                for c in range(N_CH):
                    trp = ps_tr.tile([128, SB], F32, tag="tr")
                    for t in range(N_BT):
                        nc.tensor.transpose(
                            out=trp[:, t * 128 : (t + 1) * 128],
                            in_=x_t[t][:, c * 128 : (c + 1) * 128],
                            identity=ident,
                        )
                    xc = xtp.tile([128, SB], F32R, tag="xT")
                    nc.vector.tensor_copy(out=xc, in_=trp)
                    xT.append(xc)

                # ---- h.T per feature tile; square(+bi); accumulate ||h||^2 ----
                hsq = []
                for jt in range(N_CH):
                    hp = ps_h.tile([128, SB], F32, tag="h")
                    for c in range(N_CH):
                        nc.tensor.matmul(
                            out=hp,
                            lhsT=wit_sb[:, c, jt * 128 : (jt + 1) * 128],
                            rhs=xT[c],
                            start=(c == 0),
                            stop=(c == N_CH - 1),
                        )
                    hs = hsqp.tile([128, SB], F32R, tag="hsq")
                    nc.scalar.activation(
                        out=hs, in_=hp, func=AF.Square, bias=bi_sb[:, jt : jt + 1]
                    )
                    hsq.append(hs)

                # s_un.T [P, SB] = sum_c Q_c.T @ xT_c
                sun_ps = ps_s.tile([P, SB], F32, tag="small")
                for c in range(N_CH):
                    nc.tensor.matmul(
                        out=sun_ps,
                        lhsT=q_sb[:, c, :],
                        rhs=xT[c],
                        start=(c == 0),
                        stop=(c == N_CH - 1),
                    )

                # ||h||^2 broadcast to all P partitions via ones-matmul
                nrm_ps = ps_s.tile([P, SB], F32, tag="small")
                for jt in range(N_CH):
                    nc.tensor.matmul(
                        out=nrm_ps,
                        lhsT=ones_mP,
                        rhs=hsq[jt],
                        start=(jt == 0),
                        stop=(jt == N_CH - 1),
                    )

                # rn = 10 / ||h||  (sqrt(normsq/100) then reciprocal)
                hn = smallp.tile([P, SB], F32, tag="hn")
                nc.scalar.activation(
                    out=hn, in_=nrm_ps, func=AF.Sqrt, scale=TEMP * TEMP
                )
                rn = smallp.tile([P, SB], F32, tag="rn")
                nc.vector.reciprocal(out=rn, in_=hn)

                # sim.T = (s_un + c) * rn  -> exp
                sT = smallp.tile([P, SB], F32, tag="sT")
                nc.vector.tensor_scalar(
                    out=sT, in0=sun_ps, scalar1=c_sb, scalar2=None, op0=ALU.add
                )
                simT = smallp.tile([P, SB], F32, tag="simT")
                nc.vector.tensor_mul(out=simT, in0=sT, in1=rn)
                expT = smallp.tile([P, SB], F32R, tag="expT")
                nc.scalar.activation(out=expT, in_=simT, func=AF.Exp)

                # Z = sum_p exp (broadcast to all P partitions) -> 1/Z -> attn.T
                z_ps = ps_s.tile([P, SB], F32, tag="small")
                nc.tensor.matmul(
                    out=z_ps,
                    lhsT=ones_z,
                    rhs=expT,
                    start=True,
                    stop=True,
                )
                rz = smallp.tile([P, SB], F32, tag="rz")
                nc.vector.reciprocal(out=rz, in_=z_ps)
                attnT = smallp.tile([P, SB], F32R, tag="attnT")
                nc.vector.tensor_mul(out=attnT, in0=expT.bitcast(F32), in1=rz)

                # ---- out2 = x @ Wox.T + attn @ Mp ; gelu; +x; layernorm ----
                for t in range(N_BT):
                    yt = yp.tile([128, D], F32, tag="y")
                    for jh in range(2):
                        op = ps_o.tile([128, 512], F32, tag="o")
                        for c in range(N_CH):
                            nc.tensor.matmul(
                                out=op,
                                lhsT=xT[c][:, t * 128 : (t + 1) * 128],
                                rhs=woxt_sb[:, c, jh * 512 : (jh + 1) * 512],
                                start=(c == 0),
                                stop=False,
                            )
                        nc.tensor.matmul(
                            out=op,
                            lhsT=attnT[:, t * 128 : (t + 1) * 128],
                            rhs=mp_sb[:, jh * 512 : (jh + 1) * 512],
                            start=False,
                            stop=True,
                        )
                        nc.scalar.activation(
                            out=yt[:, jh * 512 : (jh + 1) * 512], in_=op, func=AF.Gelu
                        )
                    nc.vector.tensor_add(out=yt, in0=yt, in1=x_t[t])

                    stats = statp.tile([128, 2, 6], F32, tag="stats")
                    yr = yt.rearrange("p (s f) -> p s f", f=512)
                    for s in range(2):
                        nc.vector.bn_stats(out=stats[:, s, :], in_=yr[:, s, :])
                    mv = statp.tile([128, 2], F32, tag="mv")
                    nc.vector.bn_aggr(out=mv, in_=stats)
                    sd = statp.tile([128, 1], F32, tag="sd")
                    nc.scalar.activation(
                        out=sd, in_=mv[:, 1:2], func=AF.Sqrt, bias=eps_sb
                    )
                    rstd = statp.tile([128, 1], F32, tag="rstd")
                    nc.vector.reciprocal(out=rstd, in_=sd)
                    nc.vector.tensor_scalar(
                        out=yt,
                        in0=yt,
                        scalar1=mv[:, 0:1],
                        scalar2=rstd,
                        op0=ALU.subtract,
                        op1=ALU.mult,
                    )
                    nc.sync.dma_start(
                        out=out_d[r0 + t * 128 : r0 + (t + 1) * 128, :], in_=yt
                    )

    nc.finalize()
    return nc


_RUNNER_CACHE = {}


def _get_runner(reps=1):
    """Build the bass program once and wrap it in a cached jitted callable
    (same lowering as bass2jax.run_bass_via_pjrt, reusable across calls)."""
    if reps in _RUNNER_CACHE:
        return _RUNNER_CACHE[reps]

    import jax
    from jax.sharding import Mesh, PartitionSpec
    from jax.experimental.shard_map import shard_map
    import concourse.mybir as mb
    from concourse import bass2jax

    nc = build_kernel(reps)
    bass2jax.install_neuronx_cc_hook()

    partition_name = (
        nc.partition_id_tensor.name if nc.partition_id_tensor else None
    )
    in_names, out_names, out_avals, zero_outs = [], [], [], []
    for alloc in nc.m.functions[0].allocations:
        if not isinstance(alloc, mb.MemoryLocationSet):
            continue
        name = alloc.memorylocations[0].name
        if alloc.kind == "ExternalInput":
            if name != partition_name:
                in_names.append(name)
        elif alloc.kind == "ExternalOutput":
            shape = tuple(alloc.tensor_shape)
            dtype = mb.dt.np(alloc.dtype)
            out_names.append(name)
            out_avals.append(jax.core.ShapedArray(shape, dtype))
            zero_outs.append(np.zeros(shape, dtype))
    n_params = len(in_names)
    all_in_names = in_names + out_names
    if partition_name is not None:
        all_in_names = all_in_names + [partition_name]

    def _body(*args):
        operands = list(args)
        if partition_name is not None:
            operands.append(bass2jax.partition_id_tensor())
        outs = bass2jax._bass_exec_p.bind(
            *operands,
            out_avals=tuple(out_avals),
            in_names=tuple(all_in_names),
            out_names=tuple(out_names),
            lowering_input_output_aliases=(),
            sim_require_finite=True,
            sim_require_nnan=True,
            nc=nc,
        )
        return tuple(outs)

    devices = jax.devices()[:N_CORES]
    mesh = Mesh(np.asarray(devices), ("core",))
    specs = (PartitionSpec("core"),) * (n_params + len(out_names))
    fn = jax.jit(
        shard_map(
            _body,
            mesh=mesh,
            in_specs=specs,
            out_specs=(PartitionSpec("core"),) * len(out_names),
            check_rep=False,
        ),
        donate_argnums=tuple(range(n_params, n_params + len(out_names))),
        keep_unused=True,
    )
    _RUNNER_CACHE[reps] = (fn, in_names, out_names, out_avals, zero_outs, mesh)
    return _RUNNER_CACHE[reps]


def _host_prep(Wi, bi, Wo, bo, gamma, beta, prototypes):
    f64 = np.float64
    pn = prototypes.astype(f64)
    pn = pn / np.maximum(np.linalg.norm(pn, axis=-1, keepdims=True), 1e-12)
    Q = (Wi.astype(f64).T @ pn.T).astype(np.float32)          # [D, P]
    cvec = (pn @ bi.astype(f64)).astype(np.float32)           # [P]
    Mp = (
        prototypes.astype(f64) @ Wo.astype(f64)[:, D:].T + bo.astype(f64)[None, :]
    ).astype(np.float32)                                       # [P, D]
    WiT = np.ascontiguousarray(Wi.T)                           # [D, D]
    WoxT = np.ascontiguousarray(Wo[:, :D].T)                   # [D, D]

    def chunked(w, width):
        # [D, width] -> [128, N_CH, width] with row d = c*128 + p at [p, c]
        return np.ascontiguousarray(
            w.reshape(N_CH, 128, width).transpose(1, 0, 2)
        )

    return {
        "wit": chunked(WiT, D),
        "woxt": chunked(WoxT, D),
        "q": chunked(Q, P),
        "mp": np.ascontiguousarray(Mp),
        "bi": np.ascontiguousarray(bi.reshape(N_CH, 128).T),
        "cvec": np.ascontiguousarray(cvec.reshape(P, 1)),
        "ones": np.ones((128, P), np.float32),
    }


def _concat_inputs(x, weights):
    """Build the concatenated (n_cores*dim0) operand list in in_names order."""
    fn, in_names, out_names, out_avals, zero_outs, mesh = _get_runner()
    per_core_x = x.reshape(N_CORES, BL, D)
    ops = []
    for name in in_names:
        if name == "x":
            ops.append(np.ascontiguousarray(per_core_x.reshape(N_CORES * BL, D)))
        else:
            w = weights[name]
            ops.append(
                np.ascontiguousarray(
                    np.broadcast_to(w[None], (N_CORES, *w.shape)).reshape(
                        N_CORES * w.shape[0], *w.shape[1:]
                    )
                )
            )
    zeros = [
        np.zeros((N_CORES * z.shape[0], *z.shape[1:]), z.dtype) for z in zero_outs
    ]
    return ops, zeros


def kernel(x, Wi, bi, Wo, bo, gamma, beta, prototypes):
    x = np.ascontiguousarray(np.asarray(x, dtype=np.float32))
    weights = _host_prep(
        np.asarray(Wi, np.float32), np.asarray(bi, np.float32),
        np.asarray(Wo, np.float32), np.asarray(bo, np.float32),
        np.asarray(gamma, np.float32), np.asarray(beta, np.float32),
        np.asarray(prototypes, np.float32),
    )
    fn, in_names, out_names, out_avals, zero_outs, mesh = _get_runner()
    ops, zeros = _concat_inputs(x, weights)
    out_arrs = fn(*ops, *zeros)
    out = np.asarray(out_arrs[out_names.index("out")])
    # gamma/beta are identity in this problem's setup; guard anyway.
    gamma = np.asarray(gamma, np.float32)
    beta = np.asarray(beta, np.float32)
    if not (np.all(gamma == 1.0) and np.all(beta == 0.0)):
        out = out * gamma[None, :] + beta[None, :]
    return out
